# revision 21
# baseline (speedup 1.0000x reference)
"""Trainium2 Bass kernel for nn_Attention_20925080666453.

Computation (faithful to the torch module quirk):
    e = (Q @ K) / sqrt(512)            # [B,H,S,S]
    a = softmax(e, axis=1)             # softmax over the HEAD axis
    o = a @ V                          # [B,H,S,d]
    out = o.reshape(B, S, H*d)

Sharding: 8 cores = batch (2) x query-chunk (4). The head-axis softmax couples
all 8 heads for a fixed (b, s, t), so every core keeps all heads for its query
chunk; no collectives are needed. K and V for the core's batch are duplicated
across the 4 query-chunk cores.

Per-core layout (b fixed, s_chunk of 1024 queries):
  - scores computed transposed: e_T[t, s] with lhsT = K[d, t-tile] (stationary)
    and rhs = Q^T[d, s-block] (streaming), PSUM [t=128, s=512], one bank/head.
  - exp on ScalarE directly from PSUM (scale fused), fp16 out to SBUF, into a
    GROUP tile covering 4 consecutive t-tiles.
  - softmax over heads runs once per 4-tile group as wide DVE instructions
    (tree-sum fp16 2x + fp16 reciprocal_approx_fast + broadcast normalize),
    amortizing DVE dispatch overhead; DVE is the bottleneck engine.
  - o_T[d, s] += V[t-tile, d].T @ a_T[t-tile, s-block] accumulated over the
    32 t-tiles in PSUM; AV for group g is emitted after the group g+1 QK wave
    so the in-order PE queue never stalls on the DVE normalize chain.
  - oacc drained on ScalarE to fp16; sb0 drains are interleaved into sb1's
    exp stream late enough that their AV dependencies are already retired.
"""

import os
import sys
import threading

sys.path.insert(0, "/opt/trn_rl_repo")

import numpy as np

import concourse.bacc as bacc
import concourse.bass as bass
import concourse.mybir as mybir
import concourse.tile as tile
from concourse.bass_utils import run_bass_kernel_spmd

# Problem dims
B, H, S, D = 2, 8, 4096, 64
HIDDEN = H * D
SCALE = float(1.0 / np.sqrt(np.float32(HIDDEN)))

P = 128              # partitions
NPAIR = H // 2       # head pairs
N_CORES = 8
S_CHUNKS = 4         # query chunks per batch
S_LOC = S // S_CHUNKS    # 1024 queries per core
SBLK = 512               # s-block (one PSUM bank of fp32)
NSB = S_LOC // SBLK      # 2
NTT = S // P             # 32 key tiles of 128
GRP = 4                  # t-tiles per DVE softmax group
NGRP = NTT // GRP        # 8 groups per s-block
KCH = 4                  # K dma chunks per pair (1024 keys each)
VCH = 2                  # V dma chunks per head (16 t-tiles each)

_cache = {"nc": None}
_lock = threading.Lock()


def _build():
    nc = bacc.Bacc(
        "TRN2",
        target_bir_lowering=False,
        debug=False,
        enable_asserts=True,
        num_devices=N_CORES,
    )
    f32 = mybir.dt.float32
    f16 = mybir.dt.float16

    qt_d = nc.dram_tensor("QT", [H, D, S_LOC], f16, kind="ExternalInput").ap()
    k_d = nc.dram_tensor("K", [H, D, S], f16, kind="ExternalInput").ap()
    v_d = nc.dram_tensor("V", [H, P, NTT, D], f16, kind="ExternalInput").ap()
    out_d = nc.dram_tensor("OUT", [NPAIR, P, S_LOC], f16, kind="ExternalOutput").ap()

    Exp = mybir.ActivationFunctionType.Exp

    from concourse.dve_ops import (
        RECIP_APPROX_FAST_CONSTS as _RC,
        RECIPROCAL_APPROX_FAST as _RAF,
    )

    with tile.TileContext(nc) as tc:
        with (
            tc.tile_pool(name="consts", bufs=1) as consts,
            tc.tile_pool(name="score", bufs=2, space="PSUM") as score_pool,
            tc.tile_pool(name="oaccp", bufs=1, space="PSUM") as oacc_pool,
            tc.tile_pool(name="expp", bufs=3) as exp_pool,
            tc.tile_pool(name="tmp", bufs=1) as tmp_pool,
            tc.tile_pool(name="outp", bufs=4) as outp,
        ):
            k_sb = [
                consts.tile([P, S], f16, name=f"k_sb{pr}") for pr in range(NPAIR)
            ]
            qt_sb = [
                consts.tile([P, S_LOC], f16, name=f"qt_sb{pr}")
                for pr in range(NPAIR)
            ]
            v_sb = [
                consts.tile([P, NTT, D], f16, name=f"v_sb{h}") for h in range(H)
            ]

            # Input DMAs, all on the sync HWDGE queue, priority-ordered:
            # (K chunk0 + QT sb0-half) per pair feeds the first QK wave,
            # V (whole heads) feeds the first AV wave, the rest streams
            # behind compute. Keeping the scalar queue DMA-free matters:
            # dma_start blocks on ring credits once ~16 transfers are in
            # flight, which would stall the exp stream for ~20us.
            # small critical first chunk (512 keys) so the first QK wave's
            # DMA dependency is ~1MB total; larger chunks stream behind
            kb = [0, 512, 1536, 2560, 3584, 4096]
            for pr in range(NPAIR):
                nc.sync.dma_start(
                    out=k_sb[pr][:, kb[0] : kb[1]],
                    in_=k_d[2 * pr : 2 * pr + 2, :, kb[0] : kb[1]].rearrange(
                        "h d t -> (h d) t"
                    ),
                )
                nc.sync.dma_start(
                    out=qt_sb[pr][:, 0:SBLK],
                    in_=qt_d[2 * pr : 2 * pr + 2, :, 0:SBLK].rearrange(
                        "h d s -> (h d) s"
                    ),
                )
            for h in range(H):
                nc.sync.dma_start(out=v_sb[h], in_=v_d[h])
            for c in range(1, len(kb) - 1):
                for pr in range(NPAIR):
                    nc.sync.dma_start(
                        out=k_sb[pr][:, kb[c] : kb[c + 1]],
                        in_=k_d[
                            2 * pr : 2 * pr + 2, :, kb[c] : kb[c + 1]
                        ].rearrange("h d t -> (h d) t"),
                    )
            for pr in range(NPAIR):
                nc.sync.dma_start(
                    out=qt_sb[pr][:, SBLK:S_LOC],
                    in_=qt_d[2 * pr : 2 * pr + 2, :, SBLK:S_LOC].rearrange(
                        "h d s -> (h d) s"
                    ),
                )

            def emit_av_tile(eq, g, oacc, q):
                tt = g * GRP + q
                for pr in range(NPAIR):
                    for j in range(2):
                        h = 2 * pr + j
                        nc.tensor.matmul(
                            out=oacc[pr][j * D : (j + 1) * D, :],
                            lhsT=v_sb[h][:, tt, :],
                            rhs=eq[:, q, h, :],
                            start=(tt == 0),
                            stop=(tt == NTT - 1),
                        )

            def emit_drain(pr, sb, oacc):
                ot = outp.tile([P, SBLK], f16, name="ot")
                nc.scalar.copy(out=ot, in_=oacc[pr])
                nc.sync.dma_start(
                    out=out_d[pr, :, sb * SBLK : (sb + 1) * SBLK], in_=ot
                )

            def chain_span(eq, qlo, z4q, z2q, z1q, rcpq):
                # softmax chain over a 2-tile half-group: used during the
                # startup ramp so the DVE gets work at half-group boundaries
                # instead of idling for a full group's 16 exps
                sl = slice(qlo, qlo + 2)
                nc.vector.tensor_add(
                    z4q[:, sl], eq[:, sl, 0:4, :], eq[:, sl, 4:8, :]
                )
                nc.vector.tensor_add(
                    z2q[:, sl], z4q[:, sl, 0:2, :], z4q[:, sl, 2:4, :]
                )
                nc.vector.tensor_add(
                    z1q[:, sl], z2q[:, sl, 0, :], z2q[:, sl, 1, :]
                )
                nc.vector._custom_dve(
                    _RAF, out=rcpq[:, sl], in0=z1q[:, sl],
                    s0=_RC["s0"], s1=_RC["s1"], imm2=_RC["imm2"],
                )
                rs = rcpq[:, sl]
                rcp_b = bass.AP(
                    tensor=rs.tensor,
                    offset=rs.offset,
                    ap=[rs.ap[0], rs.ap[1], [0, H], rs.ap[2]],
                )
                nc.vector.tensor_mul(eq[:, sl], eq[:, sl], rcp_b)

            def chain_tile(eq, q, z4q, z2q, z1q, rcpq):
                # per-tile softmax chain on slices of the quad tmp tiles
                nc.vector.tensor_add(
                    z4q[:, q, :, :], eq[:, q, 0:4, :], eq[:, q, 4:8, :]
                )
                nc.vector.tensor_add(
                    z2q[:, q, :, :], z4q[:, q, 0:2, :], z4q[:, q, 2:4, :]
                )
                nc.vector.tensor_add(
                    z1q[:, q, :], z2q[:, q, 0, :], z2q[:, q, 1, :]
                )
                nc.vector._custom_dve(
                    _RAF, out=rcpq[:, q, :], in0=z1q[:, q, :],
                    s0=_RC["s0"], s1=_RC["s1"], imm2=_RC["imm2"],
                )
                rcp1 = rcpq[:, q, :]
                rcp_b = bass.AP(
                    tensor=rcp1.tensor,
                    offset=rcp1.offset,
                    ap=[rcp1.ap[0], [0, H], rcp1.ap[1]],
                )
                nc.vector.tensor_mul(eq[:, q], eq[:, q], rcp_b)

            # `av_queue` carries grouped chains' (eq, g, oacc) whose AV
            # matmuls are interleaved tile-by-tile into the QK stream TWO
            # groups later. The 2-group lag matters: a group's normalize-mul
            # finishes at the end of the next wall-period, so AV emitted only
            # one group later still blocks the in-order PE queue and starves
            # the exp stream; two groups later it is always ready to run.
            av_queue = []
            oacc_prev = None
            for sb in range(NSB):
                oacc = [
                    oacc_pool.tile([P, SBLK], f32, name=f"oacc{pr}")
                    for pr in range(NPAIR)
                ]
                for g in range(NGRP):
                    # first group (cold pipeline) and the last two of the
                    # final s-block (tail + keeps the stop-flag matmul
                    # chronologically last in the PSUM accumulation) run
                    # per-tile with immediate AV
                    grouped = not (
                        (sb == 0 and g == 0)
                        or (sb == NSB - 1 and g >= NGRP - 2)
                    )
                    halves = sb == 0 and g in (1, 2, 3)
                    final_tail = sb == NSB - 1 and g >= NGRP - 2
                    emitting = (
                        av_queue.pop(0)
                        if (len(av_queue) >= 2 or (final_tail and av_queue))
                        else None
                    )
                    eq = exp_pool.tile([P, GRP, H, SBLK], f16, name="eq")
                    z4q = tmp_pool.tile([P, GRP, 4, SBLK], f16, name="z4q")
                    z2q = tmp_pool.tile([P, GRP, 2, SBLK], f16, name="z2q")
                    z1q = tmp_pool.tile([P, GRP, SBLK], f16, name="z1q")
                    rcpq = tmp_pool.tile([P, GRP, SBLK], f16, name="rcpq")
                    for q in range(GRP):
                        tt = g * GRP + q
                        for pr in range(NPAIR):
                            ps = score_pool.tile([P, 2, SBLK], f32, name="score")
                            for j in range(2):
                                # e_T[t, s] head 2*pr+j; j=1 on PE rows 64-127
                                nc.tensor.matmul(
                                    out=ps[:, j, :],
                                    lhsT=k_sb[pr][
                                        j * D : (j + 1) * D, tt * P : (tt + 1) * P
                                    ],
                                    rhs=qt_sb[pr][
                                        j * D : (j + 1) * D,
                                        sb * SBLK : (sb + 1) * SBLK,
                                    ],
                                    start=True,
                                    stop=True,
                                )
                            nc.scalar.activation(
                                out=eq[:, q, 2 * pr : 2 * pr + 2, :],
                                in_=ps[:, :, :],
                                func=Exp,
                                scale=SCALE,
                            )
                        if emitting is not None:
                            emit_av_tile(emitting[0], emitting[1], emitting[2], q)
                        # previous s-block's drains, placed after the cross-sb
                        # AV tiles retired (no scalar queue stall)
                        if sb > 0 and g == 2 and oacc_prev is not None:
                            emit_drain(q, sb - 1, oacc_prev)
                        if not grouped:
                            chain_tile(eq, q, z4q, z2q, z1q, rcpq)
                            emit_av_tile(eq, g, oacc, q)
                        elif halves and q in (1, 3):
                            chain_span(eq, q - 1, z4q, z2q, z1q, rcpq)
                    if grouped and halves:
                        av_queue.append((eq, g, oacc))
                    elif grouped:
                        # group softmax chain on DVE: Z = sum_h exp_h; a = e/Z
                        nc.vector.tensor_add(
                            z4q, eq[:, :, 0:4, :], eq[:, :, 4:8, :]
                        )
                        nc.vector.tensor_add(
                            z2q, z4q[:, :, 0:2, :], z4q[:, :, 2:4, :]
                        )
                        nc.vector.tensor_add(
                            z1q, z2q[:, :, 0, :], z2q[:, :, 1, :]
                        )
                        # custom-DVE recip; fp16 in/out (read stage converts
                        # to f32 for the bit-trick seed; write port converts)
                        nc.vector._custom_dve(
                            _RAF, out=rcpq, in0=z1q, s0=_RC["s0"], s1=_RC["s1"],
                            imm2=_RC["imm2"],
                        )
                        # normalize IN PLACE, r broadcast over the head axis
                        rcp_b = bass.AP(
                            tensor=rcpq.tensor,
                            offset=rcpq.offset,
                            ap=[rcpq.ap[0], rcpq.ap[1], [0, H], rcpq.ap[2]],
                        )
                        nc.vector.tensor_mul(eq, eq, rcp_b)
                        av_queue.append((eq, g, oacc))
                oacc_prev = oacc
            assert not av_queue, f"unemitted AV groups: {len(av_queue)}"
            # final s-block drains
            for pr in range(NPAIR):
                emit_drain(pr, NSB - 1, oacc_prev)

    nc.compile()
    return nc


def _get_nc():
    with _lock:
        if _cache["nc"] is None:
            _cache["nc"] = _build()
        return _cache["nc"]


def _prep_inputs(Q, K, V):
    Q = np.asarray(Q, dtype=np.float32)
    K = np.asarray(K, dtype=np.float32)
    V = np.asarray(V, dtype=np.float32)
    # Q^T per head: [B, H, D, S], fp16 for full-rate PE streaming
    qt = np.ascontiguousarray(Q.transpose(0, 1, 3, 2)).astype(np.float16)
    kb = K.astype(np.float16)
    # V pre-swizzled to SBUF layout [B, H, p, t_tile, d], fp16
    vp = np.ascontiguousarray(
        V.reshape(B, H, NTT, P, D).transpose(0, 1, 3, 2, 4)
    ).astype(np.float16)
    in_maps = []
    for c in range(N_CORES):
        b, sc = divmod(c, S_CHUNKS)
        in_maps.append(
            {
                "QT": np.ascontiguousarray(
                    qt[b, :, :, sc * S_LOC : (sc + 1) * S_LOC]
                ),
                "K": np.ascontiguousarray(kb[b]),
                "V": vp[b],
            }
        )
    return in_maps


def _assemble(results):
    # The reference output is a RAW reshape of contiguous [B, H, S, d] to
    # [B, S, H*d] (torch .view quirk), NOT a head-transpose. So build
    # o[B, H, S, d] and reshape.
    o_full = np.empty((B, H, S, D), dtype=np.float32)
    for c in range(N_CORES):
        b, sc = divmod(c, S_CHUNKS)
        shard = results[c]["OUT"].astype(np.float32)  # [NPAIR, 128, S_LOC]
        o_full[b, :, sc * S_LOC : (sc + 1) * S_LOC, :] = (
            shard.reshape(NPAIR, 2, D, S_LOC).transpose(0, 1, 3, 2).reshape(
                H, S_LOC, D
            )
        )
    return o_full.reshape(B, S, HIDDEN)


def run(Q, K, V, trace=False, **run_kwargs):
    nc = _get_nc()
    in_maps = _prep_inputs(Q, K, V)
    res = run_bass_kernel_spmd(
        nc, in_maps, core_ids=list(range(N_CORES)), trace=trace, **run_kwargs
    )
    return _assemble(res.results), res


def kernel(Q, K, V):
    # Force the no-trace path: the NTFF profile hook is not wired up in a
    # bare environment, and BASS_TRACE in the ambient env would crash.
    prev = os.environ.get("BASS_NEVER_TRACE")
    os.environ["BASS_NEVER_TRACE"] = "1"
    try:
        out, _ = run(Q, K, V, trace=False)
    finally:
        if prev is None:
            os.environ.pop("BASS_NEVER_TRACE", None)
        else:
            os.environ["BASS_NEVER_TRACE"] = prev
    return out
